# revision 59
# baseline (speedup 1.0000x reference)
"""DepLabeledGCN Trainium2 kernel — data-parallel variant (no collectives).

Each core processes ITS OWN batch with ALL 48 label matrices:
    s-phase:  sT[kc,l] chunks = per-label masked-adjacency matmuls (fp16,
              masks exact 0/1), label PAIRS fused into N=256 matmuls
    msum:     msg = sum_{l,kc} sT[kc,l] @ W_l^T[kc], 192 accumulating
              matmuls into one PSUM bank per layer
    relu(msg * 1/denom) -> next layer h (chunked DVE/Act ops)
then the 2-layer MLP (PE-transpose + packed PSUM) on the same core.

Weights: 24 MB fp16 streamed per label from HBM on ONE hw queue (per-core
DMA is ~410 GB/s aggregate; more queues only delays the early pairs).
The first R_RES labels stay SBUF-resident for layer 2.

Scheduling details (measured on hw traces):
  - sT tile keeps the PSUM layout [q,kc,l,i]; the psum->sbuf cast is two
    contiguous halves on vector + scalar concurrently (gpsimd cannot
    access PSUM).  msum runs l2-major so each matmul only depends on
    one label's weight DMA (layer 1 is DMA-starved; finer deps matter).
  - weight DMAs stay per-label for the same reason.
  - h0 cast and the layer-boundary relu are chunked per kc to shorten
    the critical path into each layer's first matmuls.
  - head: adjT+labT packed into one DMA (fewer ~615ns serial issue
    slots), gcn DMA split kc0/rest, wres 0..3 issued on the gpsimd
    queue concurrently; PE clock is pre-ramped with dummy matmuls on a
    memset tile while input DMAs land.
  - MLP: per-half PSUM tiles so vector+scalar work different banks;
    px2 accumulates iblk-major to pipeline behind the x1 relu; output
    DMA is split in two halves (gpsimd + sync queues).
"""

import sys

if '/opt/trn_rl_repo' not in sys.path:
    sys.path.insert(0, '/opt/trn_rl_repo')

import numpy as np

B, N, D, L = 8, 128, 512, 48
NCORES = 8
KC = D // 128
NUM_LAYERS = 2
R_RES = 16              # labels kept resident (fp16) for layer 2
NP = L // 2             # label pairs per layer
L8_LO = 4               # layer-1 labels >= L8_LO stream as e3m4 (x16)
L2_8LO = 16             # layer-2 labels >= L2_8LO reuse the RESIDENT e3m4
W8_SCALE = 16.0
N_WARM = 7              # PE clock-ramp dummy matmuls (256 cols each)


def _use8(ly, p):
    return (ly == 0 and 2 * p >= L8_LO) or (ly == 1 and 2 * p >= L2_8LO)

_CACHE = {}


def _build_nc():
    import concourse.bass as bass
    import concourse.mybir as mybir
    import concourse.tile as tile
    from concourse import bacc
    from concourse.masks import make_identity

    dt = mybir.dt
    f32 = dt.float32
    f16 = dt.float16
    Alu = mybir.AluOpType
    Act = mybir.ActivationFunctionType

    nc = bacc.Bacc("TRN2", target_bir_lowering=False, debug=False,
                   num_devices=NCORES)

    # packed [adjT | labT] — fp16 (0/1 adjacency and integer labels are
    # exact; halves the critical-path DMA bytes, 2x DVE mask rate)
    inpA_e = nc.dram_tensor("inpA", [N, 2 * N], f16, kind="ExternalInput").ap()
    # gcn pre-cast to fp16 on host: the DMA target IS h[0] (no cast op)
    gcn_e = nc.dram_tensor("gcn", [N, D], f16, kind="ExternalInput").ap()
    # misc: adjR (row-major adj) + b0 + b1 packed
    misc_e = nc.dram_tensor("misc", [N, N + 2 * KC], f32,
                            kind="ExternalInput").ap()
    wT_e = nc.dram_tensor("wT", [128, L, KC, D], f16, kind="ExternalInput").ap()
    # layer-1 copy of labels L8_LO..L-1, e3m4 scaled x16 (half the DMA
    # bytes; the 1/16 is folded into those pairs' sT casts)
    wT8_e = nc.dram_tensor("wT8", [128, L - L8_LO, KC, D], dt.float8e3,
                           kind="ExternalInput").ap()
    mlpw_e = nc.dram_tensor("mlpw", [128, 2, KC, D], f16,
                            kind="ExternalInput").ap()
    out_e = nc.dram_tensor("out", [128, KC, 128], f32,
                           kind="ExternalOutput").ap()

    with tile.TileContext(nc) as tc:
        with (
            # sT pools FIRST: the PE stationary-fetch tiles must sit at low
            # SBUF addresses — with them at ~200KB (top of usable SBUF) the
            # whole PE ran ~22% slower
            tc.tile_pool(name="sTa", bufs=7) as sTa_pool,
            tc.tile_pool(name="sTb", bufs=7) as sTb_pool,
            tc.tile_pool(name="const", bufs=1) as cpool,
            tc.tile_pool(name="spsa", bufs=3, space="PSUM") as spsa,
            tc.tile_pool(name="spsb", bufs=3, space="PSUM") as spsb,
            tc.tile_pool(name="mpsum", bufs=2, space="PSUM") as mpsum,
        ):
            # -------- PE clock pre-ramp ------------------------------------
            # dummy matmuls on a memset tile keep the tensor engine busy
            # (and its clock ramping) while the input DMAs are in flight.
            warm = cpool.tile([128, 256], f16, tag="warm")
            nc.gpsimd.memset(warm[:], 0.0)
            pw = spsa.tile([128, 2, 2, 128], f32, tag="spsa", name="warm_ps")
            for _ in range(N_WARM):
                nc.tensor.matmul(pw[:, 0, :, :], lhsT=warm[:, 0:128],
                                 rhs=warm[:], start=True, stop=True)

            # -------- critical-path input loads ----------------------------
            # adjT+labT land first (one packed fp16 DMA) so mask emission
            # can start; gcn (fp16) lands directly in h[0], kc0 first.
            # create ALL hot-small tiles first so they land at LOW SBUF
            # addresses; PE operand fetches from the top ~20KB of usable
            # SBUF run measurably slower
            inpA_sb = cpool.tile([128, 2 * N], f16, tag="inpA")
            hT = cpool.tile([128, KC, 128], f16, tag="hT")
            x1T = cpool.tile([128, KC, 128], f16, tag="x1T")
            x2 = cpool.tile([128, KC, 128], f32, tag="x2")
            mlpw_sb = cpool.tile([128, 2, KC, D], f16, tag="mlpw")
            identity = cpool.tile([128, 128], f32, tag="ident")
            maskT = cpool.tile([128, L, N], f16, tag="maskT")
            nc.sync.dma_start(inpA_sb[:], inpA_e)
            adjT_v = inpA_sb[:, 0:N]
            labT_v = inpA_sb[:, N:2 * N]

            # h0/h1 are matmul operands (fp16); the final h is only read by
            # the MLP transposes and stays fp32 so the transpose staging
            # tiles can be fp32 views of the idle s-phase PSUM pools
            h = [cpool.tile([128, D], f16 if ly < NUM_LAYERS else f32,
                            tag=f"h{ly}", name=f"h{ly}")
                 for ly in range(NUM_LAYERS + 1)]
            nc.sync.dma_start(h[0][:, 0:128], gcn_e[:, 0:128])
            nc.sync.dma_start(h[0][:, 128:D], gcn_e[:, 128:D])

            # resident fp16 weights. Labels 0..L8_LO-1 load now (layer 1
            # consumes them JIT); labels L8_LO..R_RES-1 are only needed by
            # layer 2 and load after layer 1's e3m4 stream (queue is FIFO;
            # everything stays on the sync queue — a second hw queue's
            # transfers steal engine slots from the weight stream).
            wres = cpool.tile([128, R_RES, KC, D], f16, tag="wres")
            nc.sync.dma_start(wres[:, 0:1], wT_e[:, 0:1])
            nc.sync.dma_start(wres[:, 1:2], wT_e[:, 1:2])
            # ALL e3m4 labels stay SBUF-resident: layer 1 streams them in
            # per pair, layer 2 reuses the same bytes — no layer-2 weight
            # stream at all (the run is chip-HBM-bound across 8 cores, so
            # bytes are the binding resource). One tile PER PAIR: slices
            # of a single big tile would WAW-chain the stream's DMAs.
            w8res = {p: cpool.tile([128, 2, KC, D], dt.float8e3,
                                   tag=f"w8r{p}", name=f"w8r{p}")
                     for p in range(L8_LO // 2, NP)}

            # -------- masks: maskT[j, l, i] = (labT == l) * adjT ------------
            # pairs 0..5 upfront; the rest interleaved into the layer-1 loop
            def emit_mask(l):
                nc.vector.scalar_tensor_tensor(
                    out=maskT[:, l, :],
                    in0=labT_v,
                    scalar=float(l),
                    in1=adjT_v,
                    op0=Alu.is_equal,
                    op1=Alu.mult,
                )

            for l in range(12):
                emit_mask(l)

            # pre-issue the first two e3m4 pairs ahead of the remaining
            # fp16 residents: the DMA issue ring is ~8 deep and early
            # transfers ramp slowly, so small critical items go first
            w8_issued = set()

            def issue_w8(p):
                if p not in w8_issued:
                    w8_issued.add(p)
                    nc.sync.dma_start(w8res[p][:],
                                      wT8_e[:, 2 * p - L8_LO:
                                            2 * p + 2 - L8_LO])

            issue_w8(2)
            nc.sync.dma_start(wres[:, 2:3], wT_e[:, 2:3])
            nc.sync.dma_start(wres[:, 3:4], wT_e[:, 3:4])
            issue_w8(3)
            misc_sb = cpool.tile([128, N + 2 * KC], f32, tag="misc")
            nc.sync.dma_start(misc_sb[:], misc_e)
            adjR_v = misc_sb[:, 0:N]
            b0_v = misc_sb[:, N:N + KC]
            b1_v = misc_sb[:, N + KC:N + 2 * KC]
            den = cpool.tile([128, 1], f32, tag="den")
            recip = cpool.tile([128, 1], f32, tag="recip")

            # -------- GCN layers --------------------------------------------
            def emit_s(ly, p):
                """s-phase for label pair p: one N=256 matmul per kc.
                kc 0/1 and kc 2/3 use SEPARATE psum+sbuf tiles so the
                vector and scalar psum->sbuf casts run truly in parallel
                (engines serialize on a shared psum tile).  For layer-1
                pairs whose weights stream as e3m4 (x16), the cast applies
                the exact 1/16 compensation."""
                ps_a = spsa.tile([128, 2, 2, 128], f32, tag="spsa",
                                 name="spsa")
                ps_b = spsb.tile([128, 2, 2, 128], f32, tag="spsb",
                                 name="spsb")
                for kc in range(KC):
                    ps = ps_a if kc < 2 else ps_b
                    nc.tensor.matmul(
                        ps[:, kc % 2, :, :],
                        lhsT=h[ly][:, kc * 128:(kc + 1) * 128],
                        rhs=maskT[:, 2 * p:2 * p + 2, :],
                        start=True, stop=True,
                    )
                sa = sTa_pool.tile([128, 2, 2, 128], f16, tag="sTa",
                                   name="sTa")
                sb = sTb_pool.tile([128, 2, 2, 128], f16, tag="sTb",
                                   name="sTb")
                if _use8(ly, p):
                    nc.vector.tensor_scalar_mul(sa[:], ps_a[:],
                                                1.0 / W8_SCALE)
                    nc.scalar.activation(sb[:], ps_b[:], Act.Copy,
                                         scale=1.0 / W8_SCALE)
                else:
                    nc.vector.tensor_copy(sa[:], ps_a[:])
                    nc.scalar.copy(sb[:], ps_b[:])
                return sa, sb

            def get_w(ly, p):
                """Weight pair p: resident fp16 (low labels) or the
                resident e3m4 slice (streamed in during layer 1, reused
                by layer 2)."""
                if _use8(ly, p):
                    if ly == 0:
                        issue_w8(p)
                    return w8res[p]
                return wres[:, 2 * p:2 * p + 2]

            # deep s-phase prefill: the PE does s-work while the early
            # weight stream ramps (the DMA queue starts slow); the 12
            # upfront masks cover exactly 6 pairs
            S_AHEAD = 6
            for ly in range(NUM_LAYERS):
                pm = mpsum.tile([128, D], f32, tag="mm", name="mm")
                sT_q = [emit_s(ly, q) for q in range(S_AHEAD)]
                for p in range(NP):
                    if ly == 0 and 2 * (p + 6) < L:
                        emit_mask(2 * (p + 6))
                        emit_mask(2 * (p + 6) + 1)
                    if p + S_AHEAD < NP:
                        sT_q.append(emit_s(ly, p + S_AHEAD))
                    w = get_w(ly, p)
                    sa, sb = sT_q[p]
                    if p < NP - 1:
                        for l2 in range(2):
                            for kc in range(KC):
                                i = (p * 2 + l2) * KC + kc
                                st = sa if kc < 2 else sb
                                nc.tensor.matmul(
                                    pm[:],
                                    lhsT=st[:, kc % 2, l2, :],
                                    rhs=w[:, l2, kc, :],
                                    start=(i == 0), stop=False,
                                )
                    else:
                        # last pair d-split, lo half first: the relu
                        # chain's kc0/kc1 deps release ~0.9us early and
                        # overlap the hi-half matmuls
                        for dh in range(2):
                            ds = slice(dh * 256, (dh + 1) * 256)
                            for l2 in range(2):
                                for kc in range(KC):
                                    st = sa if kc < 2 else sb
                                    nc.tensor.matmul(
                                        pm[:, ds],
                                        lhsT=st[:, kc % 2, l2, :],
                                        rhs=w[:, l2, kc, ds],
                                        start=False,
                                        stop=(dh == 1 and l2 == 1
                                              and kc == KC - 1),
                                        skip_group_check=True,
                                    )
                if ly == 0:
                    # rest of the fp16 residents (layer-2 only) + MLP
                    # weights: queue them behind layer 1's e3m4 stream in
                    # 6-label chunks (fewer ~615ns issue slots on sync,
                    # label 4 still lands early in layer 2)
                    for l in range(L8_LO, R_RES, 6):
                        hi = min(l + 6, R_RES)
                        nc.sync.dma_start(wres[:, l:hi], wT_e[:, l:hi])
                    nc.sync.dma_start(mlpw_sb[:], mlpw_e)
                    # identity for the MLP transposes (gpsimd, idle here);
                    # fp32 to match the fp32 h[2] transposes
                    make_identity(nc, identity[:])
                if ly == 0:
                    # den/recip emitted HERE so the vector queue never
                    # stalls on the misc DMA ahead of mask emission
                    nc.vector.tensor_reduce(den[:], adjR_v,
                                            mybir.AxisListType.X, Alu.add)
                    nc.vector.tensor_scalar_add(den[:], den[:], 1.0)
                    nc.vector.reciprocal(recip[:], den[:])
                # relu(msg * recip) -> next h (fp16), chunked per kc.
                # All on vector: cross-engine reads of the same PSUM bank
                # serialize anyway, and scalar ACTIVATE is slower.
                for kc in range(KC):
                    sl = slice(kc * 128, (kc + 1) * 128)
                    nc.vector.tensor_scalar(h[ly + 1][:, sl], pm[:, sl],
                                            recip[:], 0.0,
                                            Alu.mult, Alu.max)

            # -------- MLP ---------------------------------------------------
            # everything runs in the transposed domain: ptA/ptB, px1a/px1b,
            # px2a/px2b are HALF tiles so vector (lo half) and scalar (hi
            # half) read different PSUM banks concurrently.
            w0T_v = mlpw_sb[:, 0]
            w1T_v = mlpw_sb[:, 1]
            h_own = h[NUM_LAYERS]
            # four INDEPENDENT psum banks from the idle s-phase pools: no
            # transpose->copy->transpose group serialization, no collision
            # with the still-being-read msum accumulator. Copies on SCALAR
            # overlap the vector relu chain.
            for kc in range(KC):
                pool = spsa if kc % 2 == 0 else spsb
                t = pool.tile([128, 2, 2, 128], f32,
                              tag="spsa" if kc % 2 == 0 else "spsb",
                              name=f"pt{kc}")
                nc.tensor.transpose(t[:, 0, 0, :],
                                    h_own[:, kc * 128:(kc + 1) * 128],
                                    identity[:])
                nc.scalar.copy(hT[:, kc, :], t[:, 0, 0, :])

            px1a = spsa.tile([128, 2, 2, 128], f32, tag="spsa",
                             name="px1a")[:, 0]
            px1b = spsb.tile([128, 2, 2, 128], f32, tag="spsb",
                             name="px1b")[:, 0]
            for blk in range(KC):
                px1 = px1a if blk < 2 else px1b
                for kc in range(KC):
                    nc.tensor.matmul(
                        px1[:, blk % 2, :],
                        lhsT=w0T_v[:, kc, blk * 128:(blk + 1) * 128],
                        rhs=hT[:, kc, :],
                        start=(kc == 0), stop=(kc == KC - 1),
                    )
            # vector handles the px1a bank, scalar px1b, concurrently
            for blk in range(2):
                nc.vector.tensor_scalar(x1T[:, blk, :], px1a[:, blk, :],
                                        b0_v[:, blk:blk + 1], 0.0,
                                        Alu.add, Alu.max)
            for blk in range(2, KC):
                nc.scalar.activation(x1T[:, blk, :], px1b[:, blk % 2, :],
                                     Act.Relu, bias=b0_v[:, blk:blk + 1])

            px2a = spsa.tile([128, 2, 2, 128], f32, tag="spsa",
                             name="px2a")[:, 0]
            px2b = spsb.tile([128, 2, 2, 128], f32, tag="spsb",
                             name="px2b")[:, 0]
            # accumulation groups must stay SEQUENTIAL per PSUM bank — an
            # interleaved group's start=True wipes the sibling group's
            # partials (bank-granular pending-zero on hw)
            for oblk in range(KC):
                px2 = px2a if oblk < 2 else px2b
                # iblk 0,1 (vector-produced x1T) first within each group
                for j, iblk in enumerate((0, 1, 2, 3)):
                    nc.tensor.matmul(
                        px2[:, oblk % 2, :],
                        lhsT=w1T_v[:, iblk, oblk * 128:(oblk + 1) * 128],
                        rhs=x1T[:, iblk, :],
                        start=(j == 0), stop=(j == KC - 1),
                    )
            for blk in range(2):
                nc.vector.tensor_scalar(x2[:, blk, :], px2a[:, blk, :],
                                        b1_v[:, blk:blk + 1], 0.0,
                                        Alu.add, Alu.max)
            nc.sync.dma_start(out_e[:, 0:2, :], x2[:, 0:2, :])
            for blk in range(2, KC):
                nc.scalar.activation(x2[:, blk, :], px2b[:, blk % 2, :],
                                     Act.Relu, bias=b1_v[:, blk:blk + 1])
            nc.sync.dma_start(out_e[:, 2:KC, :], x2[:, 2:KC, :])

    nc.compile()
    return nc


def _get_nc():
    if "nc" not in _CACHE:
        _CACHE["nc"] = _build_nc()
    return _CACHE["nc"]


def kernel(gcn_inputs, word_seq_len, adj_matrix, dep_label_matrix,
           w_params, mlp_w0, mlp_b0, mlp_w1, mlp_b1, **_unused):
    from concourse.bass_utils import run_bass_kernel_spmd

    gcn = np.asarray(gcn_inputs, dtype=np.float32)
    adj = np.asarray(adj_matrix, dtype=np.float32)
    lab = np.asarray(dep_label_matrix)
    w = np.asarray(w_params, dtype=np.float32)
    w0 = np.asarray(mlp_w0, dtype=np.float32)
    w1 = np.asarray(mlp_w1, dtype=np.float32)
    b0 = np.asarray(mlp_b0, dtype=np.float32)
    b1 = np.asarray(mlp_b1, dtype=np.float32)

    import ml_dtypes

    # wT[kmod, l, kc, d] = w[l, d, kc*128+kmod]  (shared by all cores)
    wT32 = w.transpose(0, 2, 1).reshape(L, KC, 128, D).transpose(2, 0, 1, 3)
    wT32 = np.ascontiguousarray(wT32)
    wT = wT32.astype(np.float16)
    # layer-1 e3m4 copy of labels L8_LO.., scaled x16 to clear denormals
    wT8 = np.ascontiguousarray(
        (wT32[:, L8_LO:] * W8_SCALE)).astype(ml_dtypes.float8_e3m4)
    w0T = w0.T.reshape(KC, 128, D).transpose(1, 0, 2)
    w1T = w1.T.reshape(KC, 128, D).transpose(1, 0, 2)
    mlpw = np.ascontiguousarray(
        np.stack([w0T, w1T], axis=1)).astype(np.float16)   # [128, 2, KC, D]
    b0r = b0.reshape(KC, 128).T                            # [128, KC]
    b1r = b1.reshape(KC, 128).T
    labf = lab.astype(np.float32)

    in_maps = []
    for c in range(NCORES):
        inpA = np.empty((N, 2 * N), dtype=np.float16)
        inpA[:, 0:N] = adj[c].T
        inpA[:, N:2 * N] = labf[c].T
        miscc = np.empty((N, N + 2 * KC), dtype=np.float32)
        miscc[:, 0:N] = adj[c]
        miscc[:, N:N + KC] = b0r
        miscc[:, N + KC:N + 2 * KC] = b1r
        in_maps.append({
            "inpA": inpA,
            "gcn": gcn[c].astype(np.float16),
            "misc": miscc,
            "wT": wT,
            "wT8": wT8,
            "mlpw": mlpw,
        })

    nc = _get_nc()
    res = run_bass_kernel_spmd(nc, in_maps, list(range(NCORES)))

    out = np.empty((B, N, D), dtype=np.float32)
    for c in range(NCORES):
        arr = res.results[c]["out"]          # [dmod, dblk, i]
        out[c] = np.transpose(arr, (2, 1, 0)).reshape(N, D)
    return out


# revision 61
# speedup vs baseline: 1.0201x; 1.0201x over previous
"""DepLabeledGCN Trainium2 kernel — data-parallel variant (no collectives).

Each core processes ITS OWN batch with ALL 48 label matrices:
    s-phase:  sT[kc,l] chunks = per-label masked-adjacency matmuls (fp16,
              masks exact 0/1), label PAIRS fused into N=256 matmuls
    msum:     msg = sum_{l,kc} sT[kc,l] @ W_l^T[kc], 192 accumulating
              matmuls into one PSUM bank per layer
    relu(msg * 1/denom) -> next layer h (chunked DVE/Act ops)
then the 2-layer MLP (PE-transpose + packed PSUM) on the same core.

Weights: 24 MB fp16 streamed per label from HBM on ONE hw queue (per-core
DMA is ~410 GB/s aggregate; more queues only delays the early pairs).
The first R_RES labels stay SBUF-resident for layer 2.

Scheduling details (measured on hw traces):
  - sT tile keeps the PSUM layout [q,kc,l,i]; the psum->sbuf cast is two
    contiguous halves on vector + scalar concurrently (gpsimd cannot
    access PSUM).  msum runs l2-major so each matmul only depends on
    one label's weight DMA (layer 1 is DMA-starved; finer deps matter).
  - weight DMAs stay per-label for the same reason.
  - h0 cast and the layer-boundary relu are chunked per kc to shorten
    the critical path into each layer's first matmuls.
  - head: adjT+labT packed into one DMA (fewer ~615ns serial issue
    slots), gcn DMA split kc0/rest, wres 0..3 issued on the gpsimd
    queue concurrently; PE clock is pre-ramped with dummy matmuls on a
    memset tile while input DMAs land.
  - MLP: per-half PSUM tiles so vector+scalar work different banks;
    px2 accumulates iblk-major to pipeline behind the x1 relu; output
    DMA is split in two halves (gpsimd + sync queues).
"""

import sys

if '/opt/trn_rl_repo' not in sys.path:
    sys.path.insert(0, '/opt/trn_rl_repo')

import numpy as np

B, N, D, L = 8, 128, 512, 48
NCORES = 8
KC = D // 128
NUM_LAYERS = 2
R_RES = 16              # labels kept resident (fp16) for layer 2
NP = L // 2             # label pairs per layer
L8_LO = 4               # layer-1 labels >= L8_LO stream as e3m4 (x16)
L2_8LO = 16             # layer-2 labels >= L2_8LO reuse the RESIDENT e3m4
W8_SCALE = 16.0
N_WARM = 7              # PE clock-ramp dummy matmuls (256 cols each)


def _use8(ly, p):
    return (ly == 0 and 2 * p >= L8_LO) or (ly == 1 and 2 * p >= L2_8LO)

_CACHE = {}


def _build_nc():
    import concourse.bass as bass
    import concourse.mybir as mybir
    import concourse.tile as tile
    from concourse import bacc
    from concourse.masks import make_identity

    dt = mybir.dt
    f32 = dt.float32
    f16 = dt.float16
    Alu = mybir.AluOpType
    Act = mybir.ActivationFunctionType

    nc = bacc.Bacc("TRN2", target_bir_lowering=False, debug=False,
                   num_devices=NCORES)

    # packed [adjT | labT] — fp16 (0/1 adjacency and integer labels are
    # exact; halves the critical-path DMA bytes, 2x DVE mask rate)
    inpA_e = nc.dram_tensor("inpA", [N, 2 * N], f16, kind="ExternalInput").ap()
    # gcn pre-cast to fp16 on host: the DMA target IS h[0] (no cast op)
    gcn_e = nc.dram_tensor("gcn", [N, D], f16, kind="ExternalInput").ap()
    # misc: adjR (row-major adj) + b0 + b1 packed
    misc_e = nc.dram_tensor("misc", [N, N + 2 * KC], f32,
                            kind="ExternalInput").ap()
    wT_e = nc.dram_tensor("wT", [128, L, KC, D], f16, kind="ExternalInput").ap()
    # layer-1 copy of labels L8_LO..L-1, e3m4 scaled x16 (half the DMA
    # bytes; the 1/16 is folded into those pairs' sT casts)
    wT8_e = nc.dram_tensor("wT8", [128, L - L8_LO, KC, D], dt.float8e3,
                           kind="ExternalInput").ap()
    mlpw_e = nc.dram_tensor("mlpw", [128, 2, KC, D], f16,
                            kind="ExternalInput").ap()
    out_e = nc.dram_tensor("out", [128, KC, 128], f32,
                           kind="ExternalOutput").ap()

    with tile.TileContext(nc) as tc:
        with (
            # sT pools FIRST: the PE stationary-fetch tiles must sit at low
            # SBUF addresses — with them at ~200KB (top of usable SBUF) the
            # whole PE ran ~22% slower
            tc.tile_pool(name="sTa", bufs=4) as sTa_pool,
            tc.tile_pool(name="sTb", bufs=4) as sTb_pool,
            tc.tile_pool(name="const", bufs=1) as cpool,
            tc.tile_pool(name="spsa", bufs=3, space="PSUM") as spsa,
            tc.tile_pool(name="spsb", bufs=3, space="PSUM") as spsb,
            tc.tile_pool(name="mpsum", bufs=2, space="PSUM") as mpsum,
        ):
            # -------- PE clock pre-ramp ------------------------------------
            # dummy matmuls on a memset tile keep the tensor engine busy
            # (and its clock ramping) while the input DMAs are in flight.
            warm = cpool.tile([128, 256], f16, tag="warm")
            nc.gpsimd.memset(warm[:], 0.0)
            pw = spsa.tile([128, 2, 2, 128], f32, tag="spsa", name="warm_ps")
            for _ in range(N_WARM):
                nc.tensor.matmul(pw[:, 0, :, :], lhsT=warm[:, 0:128],
                                 rhs=warm[:], start=True, stop=True)

            # -------- critical-path input loads ----------------------------
            # adjT+labT land first (one packed fp16 DMA) so mask emission
            # can start; gcn (fp16) lands directly in h[0], kc0 first.
            # create ALL hot-small tiles first so they land at LOW SBUF
            # addresses; PE operand fetches from the top ~20KB of usable
            # SBUF run measurably slower
            inpA_sb = cpool.tile([128, 2 * N], f16, tag="inpA")
            hT = cpool.tile([128, KC, 128], f16, tag="hT")
            x1T = cpool.tile([128, KC, 128], f16, tag="x1T")
            x2 = cpool.tile([128, KC, 128], f32, tag="x2")
            mlpw_sb = cpool.tile([128, 2, KC, D], f16, tag="mlpw")
            identity = cpool.tile([128, 128], f32, tag="ident")
            maskT = cpool.tile([128, L, N], f16, tag="maskT")
            nc.sync.dma_start(inpA_sb[:], inpA_e)
            adjT_v = inpA_sb[:, 0:N]
            labT_v = inpA_sb[:, N:2 * N]

            # h0/h1 are matmul operands (fp16); the final h is only read by
            # the MLP transposes and stays fp32 so the transpose staging
            # tiles can be fp32 views of the idle s-phase PSUM pools
            h = [cpool.tile([128, D], f16 if ly < NUM_LAYERS else f32,
                            tag=f"h{ly}", name=f"h{ly}")
                 for ly in range(NUM_LAYERS + 1)]
            nc.sync.dma_start(h[0][:, 0:128], gcn_e[:, 0:128])
            nc.sync.dma_start(h[0][:, 128:D], gcn_e[:, 128:D])

            # resident fp16 weights. Labels 0..L8_LO-1 load now (layer 1
            # consumes them JIT); labels L8_LO..R_RES-1 are only needed by
            # layer 2 and load after layer 1's e3m4 stream (queue is FIFO;
            # everything stays on the sync queue — a second hw queue's
            # transfers steal engine slots from the weight stream).
            wres = cpool.tile([128, R_RES, KC, D], f16, tag="wres")
            nc.sync.dma_start(wres[:, 0:1], wT_e[:, 0:1])
            nc.sync.dma_start(wres[:, 1:2], wT_e[:, 1:2])
            # ALL e3m4 labels stay SBUF-resident: layer 1 streams them in
            # per pair, layer 2 reuses the same bytes — no layer-2 weight
            # stream at all (the run is chip-HBM-bound across 8 cores, so
            # bytes are the binding resource). One tile PER PAIR: slices
            # of a single big tile would WAW-chain the stream's DMAs.
            w8res = {p: cpool.tile([128, 2, KC, D], dt.float8e3,
                                   tag=f"w8r{p}", name=f"w8r{p}")
                     for p in range(L8_LO // 2, NP)}

            # -------- masks: maskT[j, l, i] = (labT == l) * adjT ------------
            # pairs 0..5 upfront; the rest interleaved into the layer-1 loop
            def emit_mask(l):
                nc.vector.scalar_tensor_tensor(
                    out=maskT[:, l, :],
                    in0=labT_v,
                    scalar=float(l),
                    in1=adjT_v,
                    op0=Alu.is_equal,
                    op1=Alu.mult,
                )

            for l in range(12):
                emit_mask(l)

            # pre-issue the first two e3m4 pairs ahead of the remaining
            # fp16 residents: the DMA issue ring is ~8 deep and early
            # transfers ramp slowly, so small critical items go first
            w8_issued = set()

            def issue_w8(p):
                if p not in w8_issued:
                    w8_issued.add(p)
                    nc.sync.dma_start(w8res[p][:],
                                      wT8_e[:, 2 * p - L8_LO:
                                            2 * p + 2 - L8_LO])

            issue_w8(2)
            nc.sync.dma_start(wres[:, 2:3], wT_e[:, 2:3])
            nc.sync.dma_start(wres[:, 3:4], wT_e[:, 3:4])
            issue_w8(3)
            misc_sb = cpool.tile([128, N + 2 * KC], f32, tag="misc")
            nc.sync.dma_start(misc_sb[:], misc_e)
            adjR_v = misc_sb[:, 0:N]
            b0_v = misc_sb[:, N:N + KC]
            b1_v = misc_sb[:, N + KC:N + 2 * KC]
            den = cpool.tile([128, 1], f32, tag="den")
            recip = cpool.tile([128, 1], f32, tag="recip")

            # -------- GCN layers --------------------------------------------
            def emit_s(ly, p):
                """s-phase for label pair p: one N=256 matmul per kc.
                kc 0/1 and kc 2/3 use SEPARATE psum+sbuf tiles so the
                vector and scalar psum->sbuf casts run truly in parallel
                (engines serialize on a shared psum tile).  For layer-1
                pairs whose weights stream as e3m4 (x16), the cast applies
                the exact 1/16 compensation."""
                ps_a = spsa.tile([128, 2, 2, 128], f32, tag="spsa",
                                 name="spsa")
                ps_b = spsb.tile([128, 2, 2, 128], f32, tag="spsb",
                                 name="spsb")
                for kc in range(KC):
                    ps = ps_a if kc < 2 else ps_b
                    nc.tensor.matmul(
                        ps[:, kc % 2, :, :],
                        lhsT=h[ly][:, kc * 128:(kc + 1) * 128],
                        rhs=maskT[:, 2 * p:2 * p + 2, :],
                        start=True, stop=True,
                    )
                sa = sTa_pool.tile([128, 2, 2, 128], f16, tag="sTa",
                                   name="sTa")
                sb = sTb_pool.tile([128, 2, 2, 128], f16, tag="sTb",
                                   name="sTb")
                if _use8(ly, p):
                    nc.vector.tensor_scalar_mul(sa[:], ps_a[:],
                                                1.0 / W8_SCALE)
                    nc.scalar.activation(sb[:], ps_b[:], Act.Copy,
                                         scale=1.0 / W8_SCALE)
                else:
                    nc.vector.tensor_copy(sa[:], ps_a[:])
                    nc.scalar.copy(sb[:], ps_b[:])
                return sa, sb

            def get_w(ly, p):
                """Weight pair p: resident fp16 (low labels) or the
                resident e3m4 slice (streamed in during layer 1, reused
                by layer 2)."""
                if _use8(ly, p):
                    if ly == 0:
                        issue_w8(p)
                    return w8res[p]
                return wres[:, 2 * p:2 * p + 2]

            S_AHEAD = 2
            for ly in range(NUM_LAYERS):
                pm = mpsum.tile([128, D], f32, tag="mm", name="mm")
                sT_q = [emit_s(ly, q) for q in range(S_AHEAD)]
                for p in range(NP):
                    if ly == 0 and 2 * (p + 6) < L:
                        emit_mask(2 * (p + 6))
                        emit_mask(2 * (p + 6) + 1)
                    if p + S_AHEAD < NP:
                        sT_q.append(emit_s(ly, p + S_AHEAD))
                    w = get_w(ly, p)
                    sa, sb = sT_q[p]
                    if p < NP - 1:
                        for l2 in range(2):
                            for kc in range(KC):
                                i = (p * 2 + l2) * KC + kc
                                st = sa if kc < 2 else sb
                                nc.tensor.matmul(
                                    pm[:],
                                    lhsT=st[:, kc % 2, l2, :],
                                    rhs=w[:, l2, kc, :],
                                    start=(i == 0), stop=False,
                                )
                    else:
                        # last pair d-split, lo half first: the relu
                        # chain's kc0/kc1 deps release ~0.9us early and
                        # overlap the hi-half matmuls
                        for dh in range(2):
                            ds = slice(dh * 256, (dh + 1) * 256)
                            for l2 in range(2):
                                for kc in range(KC):
                                    st = sa if kc < 2 else sb
                                    nc.tensor.matmul(
                                        pm[:, ds],
                                        lhsT=st[:, kc % 2, l2, :],
                                        rhs=w[:, l2, kc, ds],
                                        start=False,
                                        stop=(dh == 1 and l2 == 1
                                              and kc == KC - 1),
                                        skip_group_check=True,
                                    )
                if ly == 0:
                    # rest of the fp16 residents (layer-2 only) + MLP
                    # weights: queue them behind layer 1's e3m4 stream in
                    # 6-label chunks (fewer ~615ns issue slots on sync,
                    # label 4 still lands early in layer 2)
                    for l in range(L8_LO, R_RES, 6):
                        hi = min(l + 6, R_RES)
                        nc.sync.dma_start(wres[:, l:hi], wT_e[:, l:hi])
                    nc.sync.dma_start(mlpw_sb[:], mlpw_e)
                    # identity for the MLP transposes (gpsimd, idle here);
                    # fp32 to match the fp32 h[2] transposes
                    make_identity(nc, identity[:])
                if ly == 0:
                    # den/recip emitted HERE so the vector queue never
                    # stalls on the misc DMA ahead of mask emission
                    nc.vector.tensor_reduce(den[:], adjR_v,
                                            mybir.AxisListType.X, Alu.add)
                    nc.vector.tensor_scalar_add(den[:], den[:], 1.0)
                    nc.vector.reciprocal(recip[:], den[:])
                # relu(msg * recip) -> next h (fp16), chunked per kc.
                # All on vector: cross-engine reads of the same PSUM bank
                # serialize anyway, and scalar ACTIVATE is slower.
                for kc in range(KC):
                    sl = slice(kc * 128, (kc + 1) * 128)
                    nc.vector.tensor_scalar(h[ly + 1][:, sl], pm[:, sl],
                                            recip[:], 0.0,
                                            Alu.mult, Alu.max)

            # -------- MLP ---------------------------------------------------
            # everything runs in the transposed domain: ptA/ptB, px1a/px1b,
            # px2a/px2b are HALF tiles so vector (lo half) and scalar (hi
            # half) read different PSUM banks concurrently.
            w0T_v = mlpw_sb[:, 0]
            w1T_v = mlpw_sb[:, 1]
            h_own = h[NUM_LAYERS]
            # four INDEPENDENT psum banks from the idle s-phase pools: no
            # transpose->copy->transpose group serialization, no collision
            # with the still-being-read msum accumulator. Copies on SCALAR
            # overlap the vector relu chain.
            for kc in range(KC):
                pool = spsa if kc % 2 == 0 else spsb
                t = pool.tile([128, 2, 2, 128], f32,
                              tag="spsa" if kc % 2 == 0 else "spsb",
                              name=f"pt{kc}")
                nc.tensor.transpose(t[:, 0, 0, :],
                                    h_own[:, kc * 128:(kc + 1) * 128],
                                    identity[:])
                nc.scalar.copy(hT[:, kc, :], t[:, 0, 0, :])

            px1a = spsa.tile([128, 2, 2, 128], f32, tag="spsa",
                             name="px1a")[:, 0]
            px1b = spsb.tile([128, 2, 2, 128], f32, tag="spsb",
                             name="px1b")[:, 0]
            for blk in range(KC):
                px1 = px1a if blk < 2 else px1b
                for kc in range(KC):
                    nc.tensor.matmul(
                        px1[:, blk % 2, :],
                        lhsT=w0T_v[:, kc, blk * 128:(blk + 1) * 128],
                        rhs=hT[:, kc, :],
                        start=(kc == 0), stop=(kc == KC - 1),
                    )
            # vector handles the px1a bank, scalar px1b, concurrently
            for blk in range(2):
                nc.vector.tensor_scalar(x1T[:, blk, :], px1a[:, blk, :],
                                        b0_v[:, blk:blk + 1], 0.0,
                                        Alu.add, Alu.max)
            for blk in range(2, KC):
                nc.scalar.activation(x1T[:, blk, :], px1b[:, blk % 2, :],
                                     Act.Relu, bias=b0_v[:, blk:blk + 1])

            px2a = spsa.tile([128, 2, 2, 128], f32, tag="spsa",
                             name="px2a")[:, 0]
            px2b = spsb.tile([128, 2, 2, 128], f32, tag="spsb",
                             name="px2b")[:, 0]
            # accumulation groups must stay SEQUENTIAL per PSUM bank — an
            # interleaved group's start=True wipes the sibling group's
            # partials (bank-granular pending-zero on hw)
            for oblk in range(KC):
                px2 = px2a if oblk < 2 else px2b
                # iblk 0,1 (vector-produced x1T) first within each group
                for j, iblk in enumerate((0, 1, 2, 3)):
                    nc.tensor.matmul(
                        px2[:, oblk % 2, :],
                        lhsT=w1T_v[:, iblk, oblk * 128:(oblk + 1) * 128],
                        rhs=x1T[:, iblk, :],
                        start=(j == 0), stop=(j == KC - 1),
                    )
            for blk in range(2):
                nc.vector.tensor_scalar(x2[:, blk, :], px2a[:, blk, :],
                                        b1_v[:, blk:blk + 1], 0.0,
                                        Alu.add, Alu.max)
            nc.sync.dma_start(out_e[:, 0:2, :], x2[:, 0:2, :])
            for blk in range(2, KC):
                nc.scalar.activation(x2[:, blk, :], px2b[:, blk % 2, :],
                                     Act.Relu, bias=b1_v[:, blk:blk + 1])
            nc.sync.dma_start(out_e[:, 2:KC, :], x2[:, 2:KC, :])

    nc.compile()
    return nc


def _get_nc():
    if "nc" not in _CACHE:
        _CACHE["nc"] = _build_nc()
    return _CACHE["nc"]


def kernel(gcn_inputs, word_seq_len, adj_matrix, dep_label_matrix,
           w_params, mlp_w0, mlp_b0, mlp_w1, mlp_b1, **_unused):
    from concourse.bass_utils import run_bass_kernel_spmd

    gcn = np.asarray(gcn_inputs, dtype=np.float32)
    adj = np.asarray(adj_matrix, dtype=np.float32)
    lab = np.asarray(dep_label_matrix)
    w = np.asarray(w_params, dtype=np.float32)
    w0 = np.asarray(mlp_w0, dtype=np.float32)
    w1 = np.asarray(mlp_w1, dtype=np.float32)
    b0 = np.asarray(mlp_b0, dtype=np.float32)
    b1 = np.asarray(mlp_b1, dtype=np.float32)

    import ml_dtypes

    # wT[kmod, l, kc, d] = w[l, d, kc*128+kmod]  (shared by all cores)
    wT32 = w.transpose(0, 2, 1).reshape(L, KC, 128, D).transpose(2, 0, 1, 3)
    wT32 = np.ascontiguousarray(wT32)
    wT = wT32.astype(np.float16)
    # layer-1 e3m4 copy of labels L8_LO.., scaled x16 to clear denormals
    wT8 = np.ascontiguousarray(
        (wT32[:, L8_LO:] * W8_SCALE)).astype(ml_dtypes.float8_e3m4)
    w0T = w0.T.reshape(KC, 128, D).transpose(1, 0, 2)
    w1T = w1.T.reshape(KC, 128, D).transpose(1, 0, 2)
    mlpw = np.ascontiguousarray(
        np.stack([w0T, w1T], axis=1)).astype(np.float16)   # [128, 2, KC, D]
    b0r = b0.reshape(KC, 128).T                            # [128, KC]
    b1r = b1.reshape(KC, 128).T
    labf = lab.astype(np.float32)

    in_maps = []
    for c in range(NCORES):
        inpA = np.empty((N, 2 * N), dtype=np.float16)
        inpA[:, 0:N] = adj[c].T
        inpA[:, N:2 * N] = labf[c].T
        miscc = np.empty((N, N + 2 * KC), dtype=np.float32)
        miscc[:, 0:N] = adj[c]
        miscc[:, N:N + KC] = b0r
        miscc[:, N + KC:N + 2 * KC] = b1r
        in_maps.append({
            "inpA": inpA,
            "gcn": gcn[c].astype(np.float16),
            "misc": miscc,
            "wT": wT,
            "wT8": wT8,
            "mlpw": mlpw,
        })

    nc = _get_nc()
    res = run_bass_kernel_spmd(nc, in_maps, list(range(NCORES)))

    out = np.empty((B, N, D), dtype=np.float32)
    for c in range(NCORES):
        arr = res.results[c]["out"]          # [dmod, dblk, i]
        out[c] = np.transpose(arr, (2, 1, 0)).reshape(N, D)
    return out


# revision 62
# speedup vs baseline: 1.0314x; 1.0111x over previous
"""DepLabeledGCN Trainium2 kernel — data-parallel variant (no collectives).

Each core processes ITS OWN batch with ALL 48 label matrices:
    s-phase:  sT[kc,l] chunks = per-label masked-adjacency matmuls (fp16,
              masks exact 0/1), label PAIRS fused into N=256 matmuls
    msum:     msg = sum_{l,kc} sT[kc,l] @ W_l^T[kc], 192 accumulating
              matmuls into one PSUM bank per layer
    relu(msg * 1/denom) -> next layer h (chunked DVE/Act ops)
then the 2-layer MLP (PE-transpose + packed PSUM) on the same core.

Weights: 24 MB fp16 streamed per label from HBM on ONE hw queue (per-core
DMA is ~410 GB/s aggregate; more queues only delays the early pairs).
The first R_RES labels stay SBUF-resident for layer 2.

Scheduling details (measured on hw traces):
  - sT tile keeps the PSUM layout [q,kc,l,i]; the psum->sbuf cast is two
    contiguous halves on vector + scalar concurrently (gpsimd cannot
    access PSUM).  msum runs l2-major so each matmul only depends on
    one label's weight DMA (layer 1 is DMA-starved; finer deps matter).
  - weight DMAs stay per-label for the same reason.
  - h0 cast and the layer-boundary relu are chunked per kc to shorten
    the critical path into each layer's first matmuls.
  - head: adjT+labT packed into one DMA (fewer ~615ns serial issue
    slots), gcn DMA split kc0/rest, wres 0..3 issued on the gpsimd
    queue concurrently; PE clock is pre-ramped with dummy matmuls on a
    memset tile while input DMAs land.
  - MLP: per-half PSUM tiles so vector+scalar work different banks;
    px2 accumulates iblk-major to pipeline behind the x1 relu; output
    DMA is split in two halves (gpsimd + sync queues).
"""

import sys

if '/opt/trn_rl_repo' not in sys.path:
    sys.path.insert(0, '/opt/trn_rl_repo')

import numpy as np

B, N, D, L = 8, 128, 512, 48
NCORES = 8
KC = D // 128
NUM_LAYERS = 2
R_RES = 16              # labels kept resident (fp16) for layer 2
NP = L // 2             # label pairs per layer
L8_LO = 4               # layer-1 labels >= L8_LO stream as e3m4 (x16)
L2_8LO = 16             # layer-2 labels >= L2_8LO reuse the RESIDENT e3m4
W8_SCALE = 16.0
N_WARM = 7              # PE clock-ramp dummy matmuls (256 cols each)


def _use8(ly, p):
    return (ly == 0 and 2 * p >= L8_LO) or (ly == 1 and 2 * p >= L2_8LO)

_CACHE = {}


def _build_nc():
    import concourse.bass as bass
    import concourse.mybir as mybir
    import concourse.tile as tile
    from concourse import bacc
    from concourse.masks import make_identity

    dt = mybir.dt
    f32 = dt.float32
    f16 = dt.float16
    Alu = mybir.AluOpType
    Act = mybir.ActivationFunctionType

    nc = bacc.Bacc("TRN2", target_bir_lowering=False, debug=False,
                   num_devices=NCORES)

    # packed [adjT | labT] — fp16 (0/1 adjacency and integer labels are
    # exact; halves the critical-path DMA bytes, 2x DVE mask rate)
    inpA_e = nc.dram_tensor("inpA", [N, 2 * N], f16, kind="ExternalInput").ap()
    # gcn pre-cast to fp16 on host: the DMA target IS h[0] (no cast op)
    gcn_e = nc.dram_tensor("gcn", [N, D], f16, kind="ExternalInput").ap()
    # misc: adjR (row-major adj) + b0 + b1 packed
    misc_e = nc.dram_tensor("misc", [N, N + 2 * KC], f32,
                            kind="ExternalInput").ap()
    wT_e = nc.dram_tensor("wT", [128, L, KC, D], f16, kind="ExternalInput").ap()
    # layer-1 copy of labels L8_LO..L-1, e3m4 scaled x16 (half the DMA
    # bytes; the 1/16 is folded into those pairs' sT casts)
    wT8_e = nc.dram_tensor("wT8", [128, L - L8_LO, KC, D], dt.float8e3,
                           kind="ExternalInput").ap()
    mlpw_e = nc.dram_tensor("mlpw", [128, 2, KC, D], f16,
                            kind="ExternalInput").ap()
    out_e = nc.dram_tensor("out", [128, KC, 128], f32,
                           kind="ExternalOutput").ap()

    with tile.TileContext(nc) as tc:
        with (
            # sT pools FIRST: the PE stationary-fetch tiles must sit at low
            # SBUF addresses — with them at ~200KB (top of usable SBUF) the
            # whole PE ran ~22% slower
            tc.tile_pool(name="sTa", bufs=4) as sTa_pool,
            tc.tile_pool(name="sTb", bufs=4) as sTb_pool,
            tc.tile_pool(name="const", bufs=1) as cpool,
            tc.tile_pool(name="spsa", bufs=3, space="PSUM") as spsa,
            tc.tile_pool(name="spsb", bufs=3, space="PSUM") as spsb,
            tc.tile_pool(name="mpsum", bufs=2, space="PSUM") as mpsum,
        ):
            # -------- PE clock pre-ramp ------------------------------------
            # dummy matmuls on a memset tile keep the tensor engine busy
            # (and its clock ramping) while the input DMAs are in flight.
            warm = cpool.tile([128, 256], f16, tag="warm")
            nc.gpsimd.memset(warm[:], 0.0)
            pw = spsa.tile([128, 2, 2, 128], f32, tag="spsa", name="warm_ps")
            for _ in range(N_WARM):
                nc.tensor.matmul(pw[:, 0, :, :], lhsT=warm[:, 0:128],
                                 rhs=warm[:], start=True, stop=True)

            # -------- critical-path input loads ----------------------------
            # adjT+labT land first (one packed fp16 DMA) so mask emission
            # can start; gcn (fp16) lands directly in h[0], kc0 first.
            # create ALL hot-small tiles first so they land at LOW SBUF
            # addresses; PE operand fetches from the top ~20KB of usable
            # SBUF run measurably slower
            inpA_sb = cpool.tile([128, 2 * N], f16, tag="inpA")
            hT = cpool.tile([128, KC, 128], f16, tag="hT")
            x1T = cpool.tile([128, KC, 128], f16, tag="x1T")
            x2 = cpool.tile([128, KC, 128], f32, tag="x2")
            mlpw_sb = cpool.tile([128, 2, KC, D], f16, tag="mlpw")
            identity = cpool.tile([128, 128], f32, tag="ident")
            maskT = cpool.tile([128, L, N], f16, tag="maskT")
            nc.sync.dma_start(inpA_sb[:], inpA_e)
            adjT_v = inpA_sb[:, 0:N]
            labT_v = inpA_sb[:, N:2 * N]

            # h0/h1 are matmul operands (fp16); the final h is only read by
            # the MLP transposes and stays fp32 so the transpose staging
            # tiles can be fp32 views of the idle s-phase PSUM pools
            h = [cpool.tile([128, D], f16 if ly < NUM_LAYERS else f32,
                            tag=f"h{ly}", name=f"h{ly}")
                 for ly in range(NUM_LAYERS + 1)]
            nc.sync.dma_start(h[0][:, 0:128], gcn_e[:, 0:128])
            nc.sync.dma_start(h[0][:, 128:D], gcn_e[:, 128:D])

            # resident fp16 weights. Labels 0..L8_LO-1 load now (layer 1
            # consumes them JIT); labels L8_LO..R_RES-1 are only needed by
            # layer 2 and load after layer 1's e3m4 stream (queue is FIFO;
            # everything stays on the sync queue — a second hw queue's
            # transfers steal engine slots from the weight stream).
            wres = cpool.tile([128, R_RES, KC, D], f16, tag="wres")
            nc.sync.dma_start(wres[:, 0:1], wT_e[:, 0:1])
            nc.sync.dma_start(wres[:, 1:2], wT_e[:, 1:2])
            # ALL e3m4 labels stay SBUF-resident: layer 1 streams them in
            # per pair, layer 2 reuses the same bytes — no layer-2 weight
            # stream at all (the run is chip-HBM-bound across 8 cores, so
            # bytes are the binding resource). One tile PER PAIR: slices
            # of a single big tile would WAW-chain the stream's DMAs.
            w8res = {p: cpool.tile([128, 2, KC, D], dt.float8e3,
                                   tag=f"w8r{p}", name=f"w8r{p}")
                     for p in range(L8_LO // 2, NP)}

            # -------- masks: maskT[j, l, i] = (labT == l) * adjT ------------
            # pairs 0..5 upfront; the rest interleaved into the layer-1 loop
            def emit_mask(l):
                nc.vector.scalar_tensor_tensor(
                    out=maskT[:, l, :],
                    in0=labT_v,
                    scalar=float(l),
                    in1=adjT_v,
                    op0=Alu.is_equal,
                    op1=Alu.mult,
                )

            for l in range(12):
                emit_mask(l)

            # pre-issue the first two e3m4 pairs ahead of the remaining
            # fp16 residents: the DMA issue ring is ~8 deep and early
            # transfers ramp slowly, so small critical items go first
            w8_issued = set()

            def issue_w8(p):
                if p not in w8_issued:
                    w8_issued.add(p)
                    nc.sync.dma_start(w8res[p][:],
                                      wT8_e[:, 2 * p - L8_LO:
                                            2 * p + 2 - L8_LO])

            issue_w8(2)
            nc.sync.dma_start(wres[:, 2:3], wT_e[:, 2:3])
            nc.sync.dma_start(wres[:, 3:4], wT_e[:, 3:4])
            issue_w8(3)
            misc_sb = cpool.tile([128, N + 2 * KC], f32, tag="misc")
            nc.sync.dma_start(misc_sb[:], misc_e)
            adjR_v = misc_sb[:, 0:N]
            b0_v = misc_sb[:, N:N + KC]
            b1_v = misc_sb[:, N + KC:N + 2 * KC]
            den = cpool.tile([128, 1], f32, tag="den")
            recip = cpool.tile([128, 1], f32, tag="recip")

            # -------- GCN layers --------------------------------------------
            def emit_s(ly, p):
                """s-phase for label pair p: one N=256 matmul per kc.
                kc 0/1 and kc 2/3 use SEPARATE psum+sbuf tiles so the
                vector and scalar psum->sbuf casts run truly in parallel
                (engines serialize on a shared psum tile).  For layer-1
                pairs whose weights stream as e3m4 (x16), the cast applies
                the exact 1/16 compensation."""
                ps_a = spsa.tile([128, 2, 2, 128], f32, tag="spsa",
                                 name="spsa")
                ps_b = spsb.tile([128, 2, 2, 128], f32, tag="spsb",
                                 name="spsb")
                for kc in range(KC):
                    ps = ps_a if kc < 2 else ps_b
                    nc.tensor.matmul(
                        ps[:, kc % 2, :, :],
                        lhsT=h[ly][:, kc * 128:(kc + 1) * 128],
                        rhs=maskT[:, 2 * p:2 * p + 2, :],
                        start=True, stop=True,
                    )
                sa = sTa_pool.tile([128, 2, 2, 128], f16, tag="sTa",
                                   name="sTa")
                sb = sTb_pool.tile([128, 2, 2, 128], f16, tag="sTb",
                                   name="sTb")
                if _use8(ly, p):
                    nc.vector.tensor_scalar_mul(sa[:], ps_a[:],
                                                1.0 / W8_SCALE)
                    nc.scalar.activation(sb[:], ps_b[:], Act.Copy,
                                         scale=1.0 / W8_SCALE)
                else:
                    nc.vector.tensor_copy(sa[:], ps_a[:])
                    nc.scalar.copy(sb[:], ps_b[:])
                return sa, sb

            def get_w(ly, p):
                """Weight pair p: resident fp16 (low labels) or the
                resident e3m4 slice (streamed in during layer 1, reused
                by layer 2)."""
                if _use8(ly, p):
                    if ly == 0:
                        issue_w8(p)
                    return w8res[p]
                return wres[:, 2 * p:2 * p + 2]

            S_AHEAD = 2
            for ly in range(NUM_LAYERS):
                pm = mpsum.tile([128, D], f32, tag="mm", name="mm")
                sT_q = [emit_s(ly, q) for q in range(S_AHEAD)]
                for p in range(NP):
                    if ly == 0 and 2 * (p + 6) < L:
                        emit_mask(2 * (p + 6))
                        emit_mask(2 * (p + 6) + 1)
                    if p + S_AHEAD < NP:
                        sT_q.append(emit_s(ly, p + S_AHEAD))
                    w = get_w(ly, p)
                    sa, sb = sT_q[p]
                    for l2 in range(2):
                        for kc in range(KC):
                            i = (p * 2 + l2) * KC + kc
                            st = sa if kc < 2 else sb
                            nc.tensor.matmul(
                                pm[:],
                                lhsT=st[:, kc % 2, l2, :],
                                rhs=w[:, l2, kc, :],
                                start=(i == 0), stop=(i == L * KC - 1),
                            )
                if ly == 0:
                    # rest of the fp16 residents (layer-2 only) + MLP
                    # weights: queue them behind layer 1's e3m4 stream in
                    # 6-label chunks (fewer ~615ns issue slots on sync,
                    # label 4 still lands early in layer 2)
                    for l in range(L8_LO, R_RES, 6):
                        hi = min(l + 6, R_RES)
                        nc.sync.dma_start(wres[:, l:hi], wT_e[:, l:hi])
                    nc.sync.dma_start(mlpw_sb[:], mlpw_e)
                    # identity for the MLP transposes (gpsimd, idle here);
                    # fp32 to match the fp32 h[2] transposes
                    make_identity(nc, identity[:])
                if ly == 0:
                    # den/recip emitted HERE so the vector queue never
                    # stalls on the misc DMA ahead of mask emission
                    nc.vector.tensor_reduce(den[:], adjR_v,
                                            mybir.AxisListType.X, Alu.add)
                    nc.vector.tensor_scalar_add(den[:], den[:], 1.0)
                    nc.vector.reciprocal(recip[:], den[:])
                # relu(msg * recip) -> next h (fp16), chunked per kc.
                # All on vector: cross-engine reads of the same PSUM bank
                # serialize anyway, and scalar ACTIVATE is slower.
                for kc in range(KC):
                    sl = slice(kc * 128, (kc + 1) * 128)
                    nc.vector.tensor_scalar(h[ly + 1][:, sl], pm[:, sl],
                                            recip[:], 0.0,
                                            Alu.mult, Alu.max)

            # -------- MLP ---------------------------------------------------
            # everything runs in the transposed domain: ptA/ptB, px1a/px1b,
            # px2a/px2b are HALF tiles so vector (lo half) and scalar (hi
            # half) read different PSUM banks concurrently.
            w0T_v = mlpw_sb[:, 0]
            w1T_v = mlpw_sb[:, 1]
            h_own = h[NUM_LAYERS]
            # four INDEPENDENT psum banks from the idle s-phase pools: no
            # transpose->copy->transpose group serialization, no collision
            # with the still-being-read msum accumulator. Copies on SCALAR
            # overlap the vector relu chain.
            for kc in range(KC):
                pool = spsa if kc % 2 == 0 else spsb
                t = pool.tile([128, 2, 2, 128], f32,
                              tag="spsa" if kc % 2 == 0 else "spsb",
                              name=f"pt{kc}")
                nc.tensor.transpose(t[:, 0, 0, :],
                                    h_own[:, kc * 128:(kc + 1) * 128],
                                    identity[:])
                nc.scalar.copy(hT[:, kc, :], t[:, 0, 0, :])

            px1a = spsa.tile([128, 2, 2, 128], f32, tag="spsa",
                             name="px1a")[:, 0]
            px1b = spsb.tile([128, 2, 2, 128], f32, tag="spsb",
                             name="px1b")[:, 0]
            for blk in range(KC):
                px1 = px1a if blk < 2 else px1b
                for kc in range(KC):
                    nc.tensor.matmul(
                        px1[:, blk % 2, :],
                        lhsT=w0T_v[:, kc, blk * 128:(blk + 1) * 128],
                        rhs=hT[:, kc, :],
                        start=(kc == 0), stop=(kc == KC - 1),
                    )
            # vector handles the px1a bank, scalar px1b, concurrently
            for blk in range(2):
                nc.vector.tensor_scalar(x1T[:, blk, :], px1a[:, blk, :],
                                        b0_v[:, blk:blk + 1], 0.0,
                                        Alu.add, Alu.max)
            for blk in range(2, KC):
                nc.scalar.activation(x1T[:, blk, :], px1b[:, blk % 2, :],
                                     Act.Relu, bias=b0_v[:, blk:blk + 1])

            px2a = spsa.tile([128, 2, 2, 128], f32, tag="spsa",
                             name="px2a")[:, 0]
            px2b = spsb.tile([128, 2, 2, 128], f32, tag="spsb",
                             name="px2b")[:, 0]
            # accumulation groups must stay SEQUENTIAL per PSUM bank — an
            # interleaved group's start=True wipes the sibling group's
            # partials (bank-granular pending-zero on hw)
            for oblk in range(KC):
                px2 = px2a if oblk < 2 else px2b
                # iblk 0,1 (vector-produced x1T) first within each group
                for j, iblk in enumerate((0, 1, 2, 3)):
                    nc.tensor.matmul(
                        px2[:, oblk % 2, :],
                        lhsT=w1T_v[:, iblk, oblk * 128:(oblk + 1) * 128],
                        rhs=x1T[:, iblk, :],
                        start=(j == 0), stop=(j == KC - 1),
                    )
            for blk in range(2):
                nc.vector.tensor_scalar(x2[:, blk, :], px2a[:, blk, :],
                                        b1_v[:, blk:blk + 1], 0.0,
                                        Alu.add, Alu.max)
            nc.sync.dma_start(out_e[:, 0:2, :], x2[:, 0:2, :])
            for blk in range(2, KC):
                nc.scalar.activation(x2[:, blk, :], px2b[:, blk % 2, :],
                                     Act.Relu, bias=b1_v[:, blk:blk + 1])
            nc.sync.dma_start(out_e[:, 2:KC, :], x2[:, 2:KC, :])

    nc.compile()
    return nc


def _get_nc():
    if "nc" not in _CACHE:
        _CACHE["nc"] = _build_nc()
    return _CACHE["nc"]


def kernel(gcn_inputs, word_seq_len, adj_matrix, dep_label_matrix,
           w_params, mlp_w0, mlp_b0, mlp_w1, mlp_b1, **_unused):
    from concourse.bass_utils import run_bass_kernel_spmd

    gcn = np.asarray(gcn_inputs, dtype=np.float32)
    adj = np.asarray(adj_matrix, dtype=np.float32)
    lab = np.asarray(dep_label_matrix)
    w = np.asarray(w_params, dtype=np.float32)
    w0 = np.asarray(mlp_w0, dtype=np.float32)
    w1 = np.asarray(mlp_w1, dtype=np.float32)
    b0 = np.asarray(mlp_b0, dtype=np.float32)
    b1 = np.asarray(mlp_b1, dtype=np.float32)

    import ml_dtypes

    # wT[kmod, l, kc, d] = w[l, d, kc*128+kmod]  (shared by all cores)
    wT32 = w.transpose(0, 2, 1).reshape(L, KC, 128, D).transpose(2, 0, 1, 3)
    wT32 = np.ascontiguousarray(wT32)
    wT = wT32.astype(np.float16)
    # layer-1 e3m4 copy of labels L8_LO.., scaled x16 to clear denormals
    wT8 = np.ascontiguousarray(
        (wT32[:, L8_LO:] * W8_SCALE)).astype(ml_dtypes.float8_e3m4)
    w0T = w0.T.reshape(KC, 128, D).transpose(1, 0, 2)
    w1T = w1.T.reshape(KC, 128, D).transpose(1, 0, 2)
    mlpw = np.ascontiguousarray(
        np.stack([w0T, w1T], axis=1)).astype(np.float16)   # [128, 2, KC, D]
    b0r = b0.reshape(KC, 128).T                            # [128, KC]
    b1r = b1.reshape(KC, 128).T
    labf = lab.astype(np.float32)

    in_maps = []
    for c in range(NCORES):
        inpA = np.empty((N, 2 * N), dtype=np.float16)
        inpA[:, 0:N] = adj[c].T
        inpA[:, N:2 * N] = labf[c].T
        miscc = np.empty((N, N + 2 * KC), dtype=np.float32)
        miscc[:, 0:N] = adj[c]
        miscc[:, N:N + KC] = b0r
        miscc[:, N + KC:N + 2 * KC] = b1r
        in_maps.append({
            "inpA": inpA,
            "gcn": gcn[c].astype(np.float16),
            "misc": miscc,
            "wT": wT,
            "wT8": wT8,
            "mlpw": mlpw,
        })

    nc = _get_nc()
    res = run_bass_kernel_spmd(nc, in_maps, list(range(NCORES)))

    out = np.empty((B, N, D), dtype=np.float32)
    for c in range(NCORES):
        arr = res.results[c]["out"]          # [dmod, dblk, i]
        out[c] = np.transpose(arr, (2, 1, 0)).reshape(N, D)
    return out
